# revision 14
# baseline (speedup 1.0000x reference)
"""Distributed Trainium2 kernel for a multi-head attention layer.

Problem: out = AttentionLayer(query, key, value; Wq,bq,Wk,bk,Wv,bv,Wo,bo)
  B,T,N,D,H,HD = 2,12,1024,128,8,16 ; attention runs over the N (node) axis
  independently for every (b,t) pair.

Sharding: the 24 (b,t) slabs are independent -> 3 slabs per core, no
collectives.  Each core receives its three slabs of q/k/v pre-transposed to
(D, N) layout (bf16) plus replicated pre-permuted weights, and writes its
three output slabs in (D, N) f32 layout; the host unshards with a transpose.

Per-slab device pipeline (heads at 32-aligned partitions):
  1. v projection into an interleaved layout (head vals | ones cols) so the
     PV matmul simultaneously accumulates the softmax denominator.
  2. qT/kT projections into "spread" layout (head j of group g at
     partitions 32j); biases folded into the PSUM->SBUF copy (tensor_scalar).
  3. Per (group, m-chunk): 4 heads' QK^T scores (transposed orientation,
     K=16), exp on ACT (scale fused, FD=1024), then the 8 PV matmuls
     emitted adjacently so the PE runs them 4-way col-group concurrent.
  4. Normalization: reciprocal_approx_fast on denominators, PE "spread"
     matmul broadcasts 1/s across partitions, DVE multiply.
  5. Output projection with zero-padded permuted Wo; bias folded into the
     output copy.
"""

import os
import sys

import numpy as np

sys.path.insert(0, "/opt/trn_rl_repo")

import concourse.bass as bass  # noqa: E402,F401
import concourse.tile as tile  # noqa: E402
from concourse import bacc  # noqa: E402
from concourse import mybir  # noqa: E402
from concourse._compat import with_exitstack  # noqa: E402
from concourse.tile import add_dep_helper  # noqa: E402
from concourse.bass_utils import run_bass_kernel_spmd  # noqa: E402

B, T, N, D, H, HD = 2, 12, 1024, 128, 8, 16
NCORES = 8
SLABS = (B * T) // NCORES  # 3 slabs per core
F32 = mybir.dt.float32
BF16 = mybir.dt.bfloat16
SCALE = 1.0 / np.sqrt(np.float32(HD))  # 0.25
PACKW = 3104


@with_exitstack
def _build_kernel(ctx, tc: "tile.TileContext", P: dict):
    nc = tc.nc

    const = ctx.enter_context(tc.tile_pool(name="const", bufs=1))
    inp = ctx.enter_context(tc.tile_pool(name="inp", bufs=2))
    qtp = ctx.enter_context(tc.tile_pool(name="qtp", bufs=2))
    vilp = ctx.enter_context(tc.tile_pool(name="vilp", bufs=2))
    expp = ctx.enter_context(tc.tile_pool(name="expp", bufs=6))
    attnp = ctx.enter_context(tc.tile_pool(name="attnp", bufs=2))
    rsp = ctx.enter_context(tc.tile_pool(name="rsp", bufs=2))
    outp = ctx.enter_context(tc.tile_pool(name="outp", bufs=2))
    pmm = ctx.enter_context(tc.tile_pool(name="pmm", bufs=3, space="PSUM"))
    pu = ctx.enter_context(tc.tile_pool(name="pu", bufs=2, space="PSUM"))

    # ---- constants: ONE packed DMA ----
    wpack = const.tile([D, PACKW], BF16, tag="wpack")
    nc.sync.dma_start(wpack[:], P["wpack"][:])
    wqt = [wpack[:, 0:128], wpack[:, 128:256]]
    wkt = [wpack[:, 256:384], wpack[:, 384:512]]
    wot = [wpack[:, 512:640], wpack[:, 640:768]]
    hspread = wpack[:, 768:896]
    wvt_pad = wpack[:, 896:1152]
    c256 = wpack[:, 1152:1408]
    # per-partition bias columns (spread layouts, f32 for tensor_scalar)
    bpack = const.tile([D, 8], F32, tag="bpack")
    nc.sync.dma_start(bpack[:], P["bpack"][:])
    bq_col = [bpack[:, 0:1], bpack[:, 1:2]]
    bk_col = [bpack[:, 2:3], bpack[:, 3:4]]
    bfin_col = bpack[:, 4:5]

    zbias = const.tile([D, 1], F32, tag="zbias")
    nc.vector.memset(zbias[:], 0.0)

    Exp = mybir.ActivationFunctionType.Exp
    ADD = mybir.AluOpType.add

    def load_proj_pieces(s):
        """Generator: emits load + projections for slab s in small pieces."""
        xv = inp.tile([D, N], BF16, tag="xv", name=f"xv{s}")
        nc.sync.dma_start(xv[:], P["xv"][s])
        xq = inp.tile([D, N], BF16, tag="xq", name=f"xq{s}")
        nc.sync.dma_start(xq[:], P["xq"][s])
        xk = inp.tile([D, N], BF16, tag="xk", name=f"xk{s}")
        nc.sync.dma_start(xk[:], P["xk"][s])
        vil = vilp.tile([D, 8 * 256], BF16, tag="vil", name=f"vil{s}")
        qt, kt = [], []
        yield (vil, qt, kt)
        for mc in range(8):
            ps = pmm.tile([D, N], F32, tag="mm", name=f"vp{s}_{mc}")
            nc.tensor.matmul(ps[:, 0:256], xv[:, mc * 128 : (mc + 1) * 128],
                             wvt_pad, start=True, stop=True)
            nc.vector.tensor_add(vil[:, mc * 256 : (mc + 1) * 256],
                                 ps[:, 0:256], c256)
            yield None
        for g in range(2):
            for (wt, bcol, xin, dst, tg) in (
                (wqt[g], bq_col[g], xq, qt, f"q{g}"),
                (wkt[g], bk_col[g], xk, kt, f"k{g}"),
            ):
                ps = pmm.tile([D, N], F32, tag="mm", name=f"pj{s}{tg}")
                for nh in range(2):
                    nc.tensor.matmul(ps[:, nh * 512 : (nh + 1) * 512], wt,
                                     xin[:, nh * 512 : (nh + 1) * 512],
                                     start=True, stop=True)
                t = qtp.tile([D, N], BF16, tag=tg, name=f"t{s}{tg}")
                nc.vector.tensor_scalar(t[:], ps[:], bcol, None, ADD)
                dst.append(t)
                yield None
        while True:
            yield None

    def attention_group(s, g, vil, qt, kt, interleave):
        u = [pu.tile([D, 512], F32, tag="u", name=f"u{s}{g}_{nh}")
             for nh in range(2)]
        for mc in range(8):
            exs, ex_insts = [], []
            for j in range(4):
                sc = pmm.tile([D, N], F32, tag="mm", name=f"sc{s}{g}{mc}_{j}")
                for nh in range(2):
                    nc.tensor.matmul(
                        sc[:, nh * 512 : (nh + 1) * 512],
                        kt[g][32 * j : 32 * j + 16, mc * 128 : (mc + 1) * 128],
                        qt[g][32 * j : 32 * j + 16, nh * 512 : (nh + 1) * 512],
                        start=True, stop=True, tile_position=(32 * j, 0),
                    )
                ex = expp.tile([D, N], BF16, tag="ex", name=f"ex{s}{g}{mc}_{j}")
                ei = nc.scalar.activation(ex[:], sc[:], Exp, bias=zbias[:, 0:1],
                                          scale=float(SCALE))
                exs.append(ex)
                ex_insts.append(ei)
            with tc.high_priority():
                for nh in range(2):
                    for j in range(4):
                        lo = mc * 256 + g * 128 + 32 * j
                        mm = nc.tensor.matmul(
                            u[nh][32 * j : 32 * j + 32, :],
                            vil[:, lo : lo + 32],
                            exs[j][:, nh * 512 : (nh + 1) * 512],
                            start=(mc == 0), stop=(mc == 7),
                            tile_position=(0, 32 * j))
                        add_dep_helper(mm.ins, ex_insts[3].ins,
                                       reason="PV quad grouping")
            if interleave is not None:
                next(interleave)
                next(interleave)
        return u

    def norm_group(s, g, u):
        # copy U out of PSUM first so the pool slots free early
        uc = rsp.tile([D, N], F32, tag="uc", name=f"uc{s}{g}")
        nc.vector.tensor_copy(uc[:, 0:512], u[0][:])
        nc.vector.tensor_copy(uc[:, 512:1024], u[1][:])
        rtmp = rsp.tile([D, N], F32, tag="rtmp", name=f"rt{s}{g}")
        nc.vector.reciprocal_approx_fast(rtmp[:], uc[:])
        rrec = rsp.tile([D, N], BF16, tag="rrec", name=f"rr{s}{g}")
        nc.vector.tensor_copy(rrec[:], rtmp[:])
        rps = pmm.tile([D, N], F32, tag="mm", name=f"rp{s}{g}")
        for nh in range(2):
            nc.tensor.matmul(rps[:, nh * 512 : (nh + 1) * 512], hspread,
                             rrec[:, nh * 512 : (nh + 1) * 512],
                             start=True, stop=True)
        rsb = rsp.tile([D, N], F32, tag="rsb", name=f"rb{s}{g}")
        nc.vector.tensor_copy(rsb[:], rps[:])
        a = attnp.tile([D, N], BF16, tag=f"at{g}", name=f"a{s}{g}")
        nc.vector.tensor_mul(a[:], uc[:], rsb[:])
        return a

    pipe = load_proj_pieces(0)
    cur = next(pipe)
    for _ in range(21):
        next(pipe)
    for s in range(SLABS):
        vil, qt, kt = cur
        nxt_pipe = load_proj_pieces(s + 1) if s + 1 < SLABS else None
        nxt = next(nxt_pipe) if nxt_pipe else None
        u_a = attention_group(s, 0, vil, qt, kt, None)
        at0 = norm_group(s, 0, u_a)
        u_b = attention_group(s, 1, vil, qt, kt, nxt_pipe)
        at1 = norm_group(s, 1, u_b)
        at = [at0, at1]
        cur = nxt

        # ---- output projection; bias folded into the output copy ----
        fin = pmm.tile([D, N], F32, tag="mm", name=f"fin{s}")
        for nh in range(2):
            c = fin[:, nh * 512 : (nh + 1) * 512]
            nc.tensor.matmul(c, wot[0], at[0][:, nh * 512 : (nh + 1) * 512],
                             start=True, stop=False)
            nc.tensor.matmul(c, wot[1], at[1][:, nh * 512 : (nh + 1) * 512],
                             start=False, stop=True)
        ot = outp.tile([D, N], F32, tag="ot", name=f"ot{s}")
        nc.vector.tensor_scalar(ot[:], fin[:], bfin_col, None, ADD)
        nc.sync.dma_start(P["out"][s], ot[:])


_CACHE: dict = {}


def _get_nc():
    if "nc" in _CACHE:
        return _CACHE["nc"]
    nc = bacc.Bacc()
    P = {}
    for name, shape in (
        ("xq", (SLABS, D, N)), ("xk", (SLABS, D, N)), ("xv", (SLABS, D, N)),
        ("wpack", (D, PACKW)),
    ):
        P[name] = nc.declare_dram_parameter(name, list(shape), BF16, isOutput=False)
    P["bpack"] = nc.declare_dram_parameter("bpack", [D, 8], F32, isOutput=False)
    P["out"] = nc.declare_dram_parameter("out", [SLABS, D, N], F32, isOutput=True)

    with tile.TileContext(nc) as tc:
        _build_kernel(tc, P)
    nc.finalize()
    _CACHE["nc"] = nc
    return nc


def _spread_w(W, off):
    """(128,128) lhsT for q/k projection: head j of this group at cols 32j."""
    A = np.zeros((D, D), np.float32)
    for j in range(4):
        A[:, 32 * j : 32 * j + 16] = W[off + 16 * j : off + 16 * j + 16, :].T
    return A


def _spread_b(b, off):
    r = np.zeros(D, np.float32)
    for j in range(4):
        r[32 * j : 32 * j + 16] = b[off + 16 * j : off + 16 * j + 16]
    return r


def _host_consts(Wq, bq, Wk, bk, Wv, bv, Wo, bo):
    pack = np.zeros((D, PACKW), np.float32)
    pack[:, 0:128] = _spread_w(Wq, 0)
    pack[:, 128:256] = _spread_w(Wq, 64)
    pack[:, 256:384] = _spread_w(Wk, 0)
    pack[:, 384:512] = _spread_w(Wk, 64)
    wo_a = np.zeros((D, D), np.float32)
    wo_b = np.zeros((D, D), np.float32)
    for j in range(4):
        wo_a[32 * j : 32 * j + 16, :] = Wo[:, 16 * j : 16 * j + 16].T
        wo_b[32 * j : 32 * j + 16, :] = Wo[:, 64 + 16 * j : 64 + 16 * j + 16].T
    pack[:, 512:640] = wo_a
    pack[:, 640:768] = wo_b
    hs = np.zeros((D, D), np.float32)
    for p in range(D):
        hs[32 * (p // 32) + 16, p] = 1.0
    pack[:, 768:896] = hs
    wvt = np.zeros((D, 256), np.float32)
    c256 = np.zeros((D, 256), np.float32)
    for g in range(2):
        for j in range(4):
            h = 4 * g + j
            base = g * 128 + 32 * j
            wvt[:, base : base + 16] = Wv[16 * h : 16 * h + 16, :].T
            c256[:, base + 16 : base + 32] = 1.0
    pack[:, 896:1152] = wvt
    pack[:, 1152:1408] = c256
    bp = np.zeros((D, 8), np.float32)
    bp[:, 0] = _spread_b(bq, 0)
    bp[:, 1] = _spread_b(bq, 64)
    bp[:, 2] = _spread_b(bk, 0)
    bp[:, 3] = _spread_b(bk, 64)
    bp[:, 4] = (Wo @ bv + bo).astype(np.float32)
    import ml_dtypes
    return {"wpack": pack.astype(ml_dtypes.bfloat16), "bpack": bp}


def kernel(**inputs) -> np.ndarray:
    q = np.asarray(inputs["query"], np.float32)
    k = np.asarray(inputs["key"], np.float32)
    v = np.asarray(inputs["value"], np.float32)
    consts = _host_consts(
        *(np.asarray(inputs[n], np.float32)
          for n in ("Wq", "bq", "Wk", "bk", "Wv", "bv", "Wo", "bo"))
    )
    # slabs in (D, N) layout, bf16 for full-rate PE streams
    import ml_dtypes
    bf = ml_dtypes.bfloat16
    qT = np.ascontiguousarray(q.reshape(B * T, N, D).transpose(0, 2, 1)).astype(bf)
    kT = np.ascontiguousarray(k.reshape(B * T, N, D).transpose(0, 2, 1)).astype(bf)
    vT = np.ascontiguousarray(v.reshape(B * T, N, D).transpose(0, 2, 1)).astype(bf)

    nc = _get_nc()
    in_maps = []
    for c in range(NCORES):
        sl = slice(SLABS * c, SLABS * (c + 1))
        m = {"xq": qT[sl], "xk": kT[sl], "xv": vT[sl]}
        m.update(consts)
        in_maps.append(m)

    res = run_bass_kernel_spmd(nc, in_maps, core_ids=list(range(NCORES)),
                               trace=bool(int(os.environ.get("KERNEL_TRACE", "0"))))
    _CACHE["last_result"] = res
    out = np.concatenate([res.results[c]["out"] for c in range(NCORES)], axis=0)
    return np.ascontiguousarray(
        out.transpose(0, 2, 1).reshape(B, T, N, D)).astype(np.float32)


# revision 15
# speedup vs baseline: 1.0044x; 1.0044x over previous
"""Distributed Trainium2 kernel for a multi-head attention layer.

Problem: out = AttentionLayer(query, key, value; Wq,bq,Wk,bk,Wv,bv,Wo,bo)
  B,T,N,D,H,HD = 2,12,1024,128,8,16 ; attention runs over the N (node) axis
  independently for every (b,t) pair.

Sharding: the 24 (b,t) slabs are independent -> 3 slabs per core, no
collectives.  Each core receives its three slabs of q/k/v pre-transposed to
(D, N) layout (bf16) plus replicated pre-permuted weights, and writes its
three output slabs in (D, N) f32 layout; the host unshards with a transpose.

Per-slab device pipeline (heads at 32-aligned partitions):
  1. v projection into an interleaved layout (head vals | ones cols) so the
     PV matmul simultaneously accumulates the softmax denominator.
  2. qT/kT projections into "spread" layout (head j of group g at
     partitions 32j); biases folded into the PSUM->SBUF copy (tensor_scalar).
  3. Per (group, m-chunk): 4 heads' QK^T scores (transposed orientation,
     K=16), exp on ACT (scale fused, FD=1024), then the 8 PV matmuls
     emitted adjacently so the PE runs them 4-way col-group concurrent.
  4. Normalization: reciprocal_approx_fast on denominators, PE "spread"
     matmul broadcasts 1/s across partitions, DVE multiply.
  5. Output projection with zero-padded permuted Wo; bias folded into the
     output copy.
"""

import os
import sys

import numpy as np

sys.path.insert(0, "/opt/trn_rl_repo")

import concourse.bass as bass  # noqa: E402,F401
import concourse.tile as tile  # noqa: E402
from concourse import bacc  # noqa: E402
from concourse import mybir  # noqa: E402
from concourse._compat import with_exitstack  # noqa: E402
from concourse.tile import add_dep_helper  # noqa: E402
from concourse.bass_utils import run_bass_kernel_spmd  # noqa: E402

B, T, N, D, H, HD = 2, 12, 1024, 128, 8, 16
NCORES = 8
SLABS = (B * T) // NCORES  # 3 slabs per core
F32 = mybir.dt.float32
BF16 = mybir.dt.bfloat16
SCALE = 1.0 / np.sqrt(np.float32(HD))  # 0.25
PACKW = 3104


@with_exitstack
def _build_kernel(ctx, tc: "tile.TileContext", P: dict):
    nc = tc.nc

    const = ctx.enter_context(tc.tile_pool(name="const", bufs=1))
    inp = ctx.enter_context(tc.tile_pool(name="inp", bufs=2))
    qtp = ctx.enter_context(tc.tile_pool(name="qtp", bufs=2))
    vilp = ctx.enter_context(tc.tile_pool(name="vilp", bufs=2))
    expp = ctx.enter_context(tc.tile_pool(name="expp", bufs=6))
    attnp = ctx.enter_context(tc.tile_pool(name="attnp", bufs=2))
    rsp = ctx.enter_context(tc.tile_pool(name="rsp", bufs=2))
    outp = ctx.enter_context(tc.tile_pool(name="outp", bufs=2))
    pmm = ctx.enter_context(tc.tile_pool(name="pmm", bufs=3, space="PSUM"))
    pu = ctx.enter_context(tc.tile_pool(name="pu", bufs=2, space="PSUM"))

    # ---- constants: ONE packed DMA ----
    wpack = const.tile([D, PACKW], BF16, tag="wpack")
    nc.sync.dma_start(wpack[:], P["wpack"][:])
    wqt = [wpack[:, 0:128], wpack[:, 128:256]]
    wkt = [wpack[:, 256:384], wpack[:, 384:512]]
    wot = [wpack[:, 512:640], wpack[:, 640:768]]
    hspread = wpack[:, 768:896]
    wvt_pad = wpack[:, 896:1152]
    c256 = wpack[:, 1152:1408]
    # per-partition bias columns (spread layouts, f32 for tensor_scalar)
    bpack = const.tile([D, 8], F32, tag="bpack")
    nc.sync.dma_start(bpack[:], P["bpack"][:])
    bq_col = [bpack[:, 0:1], bpack[:, 1:2]]
    bk_col = [bpack[:, 2:3], bpack[:, 3:4]]
    bfin_col = bpack[:, 4:5]

    zbias = const.tile([D, 1], F32, tag="zbias")
    nc.vector.memset(zbias[:], 0.0)

    Exp = mybir.ActivationFunctionType.Exp
    ADD = mybir.AluOpType.add

    def load_proj_pieces(s):
        """Generator: emits load + projections for slab s in small pieces."""
        xv = inp.tile([D, N], BF16, tag="xv", name=f"xv{s}")
        nc.sync.dma_start(xv[:], P["xv"][s])
        xq = inp.tile([D, N], BF16, tag="xq", name=f"xq{s}")
        nc.sync.dma_start(xq[:], P["xq"][s])
        xk = inp.tile([D, N], BF16, tag="xk", name=f"xk{s}")
        nc.sync.dma_start(xk[:], P["xk"][s])
        vil = vilp.tile([D, 8 * 256], BF16, tag="vil", name=f"vil{s}")
        qt, kt = [], []
        yield (vil, qt, kt)
        for mc in range(8):
            ps = pmm.tile([D, N], F32, tag="mm", name=f"vp{s}_{mc}")
            nc.tensor.matmul(ps[:, 0:256], xv[:, mc * 128 : (mc + 1) * 128],
                             wvt_pad, start=True, stop=True)
            nc.vector.tensor_add(vil[:, mc * 256 : (mc + 1) * 256],
                                 ps[:, 0:256], c256)
            yield None
        for g in range(2):
            for (wt, bcol, xin, dst, tg) in (
                (wqt[g], bq_col[g], xq, qt, f"q{g}"),
                (wkt[g], bk_col[g], xk, kt, f"k{g}"),
            ):
                ps = pmm.tile([D, N], F32, tag="mm", name=f"pj{s}{tg}")
                for nh in range(2):
                    nc.tensor.matmul(ps[:, nh * 512 : (nh + 1) * 512], wt,
                                     xin[:, nh * 512 : (nh + 1) * 512],
                                     start=True, stop=True)
                t = qtp.tile([D, N], BF16, tag=tg, name=f"t{s}{tg}")
                nc.vector.tensor_scalar(t[:], ps[:], bcol, None, ADD)
                dst.append(t)
                yield None
        while True:
            yield None

    def attention_group(s, g, vil, qt, kt, interleave, fills=()):
        fills = list(fills)
        u = [pu.tile([D, 512], F32, tag="u", name=f"u{s}{g}_{nh}")
             for nh in range(2)]
        for mc in range(8):
            if fills:
                fills.pop(0)()
            exs, ex_insts = [], []
            for j in range(4):
                sc = pmm.tile([D, N], F32, tag="mm", name=f"sc{s}{g}{mc}_{j}")
                for nh in range(2):
                    nc.tensor.matmul(
                        sc[:, nh * 512 : (nh + 1) * 512],
                        kt[g][32 * j : 32 * j + 16, mc * 128 : (mc + 1) * 128],
                        qt[g][32 * j : 32 * j + 16, nh * 512 : (nh + 1) * 512],
                        start=True, stop=True, tile_position=(32 * j, 0),
                    )
                ex = expp.tile([D, N], BF16, tag="ex", name=f"ex{s}{g}{mc}_{j}")
                ei = nc.scalar.activation(ex[:], sc[:], Exp, bias=zbias[:, 0:1],
                                          scale=float(SCALE))
                exs.append(ex)
                ex_insts.append(ei)
            with tc.high_priority():
                for nh in range(2):
                    for j in range(4):
                        lo = mc * 256 + g * 128 + 32 * j
                        mm = nc.tensor.matmul(
                            u[nh][32 * j : 32 * j + 32, :],
                            vil[:, lo : lo + 32],
                            exs[j][:, nh * 512 : (nh + 1) * 512],
                            start=(mc == 0), stop=(mc == 7),
                            tile_position=(0, 32 * j))
                        add_dep_helper(mm.ins, ex_insts[3].ins,
                                       reason="PV quad grouping")
            if interleave is not None:
                next(interleave)
                next(interleave)
        return u

    def norm_dve(s, g, u):
        # copy U out of PSUM first so the pool slots free early (DVE only)
        uc = rsp.tile([D, N], F32, tag="uc", name=f"uc{s}{g}")
        nc.vector.tensor_copy(uc[:, 0:512], u[0][:])
        nc.vector.tensor_copy(uc[:, 512:1024], u[1][:])
        rtmp = rsp.tile([D, N], F32, tag="rtmp", name=f"rt{s}{g}")
        nc.vector.reciprocal_approx_fast(rtmp[:], uc[:])
        rrec = rsp.tile([D, N], BF16, tag="rrec", name=f"rr{s}{g}")
        nc.vector.tensor_copy(rrec[:], rtmp[:])
        return uc, rrec

    def norm_pe(s, g, uc, rrec):
        # spread matmul + normalize; emitted where the PE has ready work
        a = attnp.tile([D, N], BF16, tag=f"at{g}", name=f"a{s}{g}")
        rps = pmm.tile([D, N], F32, tag="mm", name=f"rp{s}{g}")
        for nh in range(2):
            nc.tensor.matmul(rps[:, nh * 512 : (nh + 1) * 512], hspread,
                             rrec[:, nh * 512 : (nh + 1) * 512],
                             start=True, stop=True)
        rsb = rsp.tile([D, N], F32, tag="rsb", name=f"rb{s}{g}")
        nc.vector.tensor_copy(rsb[:], rps[:])
        nc.vector.tensor_mul(a[:], uc[:], rsb[:])
        return a

    def final_out(s, at):
        fin = pmm.tile([D, N], F32, tag="mm", name=f"fin{s}")
        for nh in range(2):
            c = fin[:, nh * 512 : (nh + 1) * 512]
            nc.tensor.matmul(c, wot[0], at[0][:, nh * 512 : (nh + 1) * 512],
                             start=True, stop=False)
            nc.tensor.matmul(c, wot[1], at[1][:, nh * 512 : (nh + 1) * 512],
                             start=False, stop=True)
        ot = outp.tile([D, N], F32, tag="ot", name=f"ot{s}")
        nc.vector.tensor_scalar(ot[:], fin[:], bfin_col, None, ADD)
        nc.sync.dma_start(P["out"][s], ot[:])

    pipe = load_proj_pieces(0)
    cur = next(pipe)
    for _ in range(21):
        next(pipe)
    carry = []  # fills deferred into the next slab's attention-A
    for s in range(SLABS):
        vil, qt, kt = cur
        nxt_pipe = load_proj_pieces(s + 1) if s + 1 < SLABS else None
        nxt = next(nxt_pipe) if nxt_pipe else None

        slabfills = {}
        u_a = attention_group(s, 0, vil, qt, kt, None, fills=carry)
        carry = []
        uc_a, rrec_a = norm_dve(s, 0, u_a)
        at_s = []
        fills_b = [lambda s=s, uc=uc_a, rr=rrec_a: at_s.append(norm_pe(s, 0, uc, rr))]
        u_b = attention_group(s, 1, vil, qt, kt, nxt_pipe, fills=fills_b)
        uc_b, rrec_b = norm_dve(s, 1, u_b)

        def mk_tail(s, uc_b, rrec_b, at_s):
            def f1():
                at_s.append(norm_pe(s, 1, uc_b, rrec_b))
            def f2():
                final_out(s, at_s)
            return [f1, f2]
        carry = mk_tail(s, uc_b, rrec_b, at_s)
        cur = nxt
    # last slab's tail has no next attention block to hide in
    for f in carry:
        f()


_CACHE: dict = {}


def _get_nc():
    if "nc" in _CACHE:
        return _CACHE["nc"]
    nc = bacc.Bacc()
    P = {}
    for name, shape in (
        ("xq", (SLABS, D, N)), ("xk", (SLABS, D, N)), ("xv", (SLABS, D, N)),
        ("wpack", (D, PACKW)),
    ):
        P[name] = nc.declare_dram_parameter(name, list(shape), BF16, isOutput=False)
    P["bpack"] = nc.declare_dram_parameter("bpack", [D, 8], F32, isOutput=False)
    P["out"] = nc.declare_dram_parameter("out", [SLABS, D, N], F32, isOutput=True)

    with tile.TileContext(nc) as tc:
        _build_kernel(tc, P)
    nc.finalize()
    _CACHE["nc"] = nc
    return nc


def _spread_w(W, off):
    """(128,128) lhsT for q/k projection: head j of this group at cols 32j."""
    A = np.zeros((D, D), np.float32)
    for j in range(4):
        A[:, 32 * j : 32 * j + 16] = W[off + 16 * j : off + 16 * j + 16, :].T
    return A


def _spread_b(b, off):
    r = np.zeros(D, np.float32)
    for j in range(4):
        r[32 * j : 32 * j + 16] = b[off + 16 * j : off + 16 * j + 16]
    return r


def _host_consts(Wq, bq, Wk, bk, Wv, bv, Wo, bo):
    pack = np.zeros((D, PACKW), np.float32)
    pack[:, 0:128] = _spread_w(Wq, 0)
    pack[:, 128:256] = _spread_w(Wq, 64)
    pack[:, 256:384] = _spread_w(Wk, 0)
    pack[:, 384:512] = _spread_w(Wk, 64)
    wo_a = np.zeros((D, D), np.float32)
    wo_b = np.zeros((D, D), np.float32)
    for j in range(4):
        wo_a[32 * j : 32 * j + 16, :] = Wo[:, 16 * j : 16 * j + 16].T
        wo_b[32 * j : 32 * j + 16, :] = Wo[:, 64 + 16 * j : 64 + 16 * j + 16].T
    pack[:, 512:640] = wo_a
    pack[:, 640:768] = wo_b
    hs = np.zeros((D, D), np.float32)
    for p in range(D):
        hs[32 * (p // 32) + 16, p] = 1.0
    pack[:, 768:896] = hs
    wvt = np.zeros((D, 256), np.float32)
    c256 = np.zeros((D, 256), np.float32)
    for g in range(2):
        for j in range(4):
            h = 4 * g + j
            base = g * 128 + 32 * j
            wvt[:, base : base + 16] = Wv[16 * h : 16 * h + 16, :].T
            c256[:, base + 16 : base + 32] = 1.0
    pack[:, 896:1152] = wvt
    pack[:, 1152:1408] = c256
    bp = np.zeros((D, 8), np.float32)
    bp[:, 0] = _spread_b(bq, 0)
    bp[:, 1] = _spread_b(bq, 64)
    bp[:, 2] = _spread_b(bk, 0)
    bp[:, 3] = _spread_b(bk, 64)
    bp[:, 4] = (Wo @ bv + bo).astype(np.float32)
    import ml_dtypes
    return {"wpack": pack.astype(ml_dtypes.bfloat16), "bpack": bp}


def kernel(**inputs) -> np.ndarray:
    q = np.asarray(inputs["query"], np.float32)
    k = np.asarray(inputs["key"], np.float32)
    v = np.asarray(inputs["value"], np.float32)
    consts = _host_consts(
        *(np.asarray(inputs[n], np.float32)
          for n in ("Wq", "bq", "Wk", "bk", "Wv", "bv", "Wo", "bo"))
    )
    # slabs in (D, N) layout, bf16 for full-rate PE streams
    import ml_dtypes
    bf = ml_dtypes.bfloat16
    qT = np.ascontiguousarray(q.reshape(B * T, N, D).transpose(0, 2, 1)).astype(bf)
    kT = np.ascontiguousarray(k.reshape(B * T, N, D).transpose(0, 2, 1)).astype(bf)
    vT = np.ascontiguousarray(v.reshape(B * T, N, D).transpose(0, 2, 1)).astype(bf)

    nc = _get_nc()
    in_maps = []
    for c in range(NCORES):
        sl = slice(SLABS * c, SLABS * (c + 1))
        m = {"xq": qT[sl], "xk": kT[sl], "xv": vT[sl]}
        m.update(consts)
        in_maps.append(m)

    res = run_bass_kernel_spmd(nc, in_maps, core_ids=list(range(NCORES)),
                               trace=bool(int(os.environ.get("KERNEL_TRACE", "0"))))
    _CACHE["last_result"] = res
    out = np.concatenate([res.results[c]["out"] for c in range(NCORES)], axis=0)
    return np.ascontiguousarray(
        out.transpose(0, 2, 1).reshape(B, T, N, D)).astype(np.float32)


# revision 16
# speedup vs baseline: 1.1162x; 1.1113x over previous
"""Distributed Trainium2 kernel for a multi-head attention layer.

Problem: out = AttentionLayer(query, key, value; Wq,bq,Wk,bk,Wv,bv,Wo,bo)
  B,T,N,D,H,HD = 2,12,1024,128,8,16 ; attention runs over the N (node) axis
  independently for every (b,t) pair.

Sharding: the 24 (b,t) slabs are independent -> 3 slabs per core, no
collectives.  Each core receives its three slabs of q/k/v pre-transposed to
(D, N) layout (bf16) plus replicated pre-permuted weights, and writes its
three output slabs in (D, N) f32 layout; the host unshards with a transpose.

Per-slab device pipeline (heads at 32-aligned partitions):
  1. v projection into an interleaved layout (head vals | ones cols) so the
     PV matmul simultaneously accumulates the softmax denominator.
  2. qT/kT projections into "spread" layout (head j of group g at
     partitions 32j); biases folded into the PSUM->SBUF copy (tensor_scalar).
  3. Per (group, m-chunk): 4 heads' QK^T scores (transposed orientation,
     K=16), exp on ACT (scale fused, FD=1024), then the 8 PV matmuls
     emitted adjacently so the PE runs them 4-way col-group concurrent.
  4. Normalization: reciprocal_approx_fast on denominators, PE "spread"
     matmul broadcasts 1/s across partitions, DVE multiply.
  5. Output projection with zero-padded permuted Wo; bias folded into the
     output copy.
"""

import os
import sys

import numpy as np

sys.path.insert(0, "/opt/trn_rl_repo")

import concourse.bass as bass  # noqa: E402,F401
import concourse.tile as tile  # noqa: E402
from concourse import bacc  # noqa: E402
from concourse import mybir  # noqa: E402
from concourse._compat import with_exitstack  # noqa: E402
from concourse.tile import add_dep_helper  # noqa: E402
from concourse.bass_utils import run_bass_kernel_spmd  # noqa: E402

B, T, N, D, H, HD = 2, 12, 1024, 128, 8, 16
NCORES = 8
SLABS = (B * T) // NCORES  # 3 slabs per core
F32 = mybir.dt.float32
BF16 = mybir.dt.bfloat16
SCALE = 1.0 / np.sqrt(np.float32(HD))  # 0.25
PACKW = 3104


@with_exitstack
def _build_kernel(ctx, tc: "tile.TileContext", P: dict):
    nc = tc.nc

    const = ctx.enter_context(tc.tile_pool(name="const", bufs=1))
    inp = ctx.enter_context(tc.tile_pool(name="inp", bufs=2))
    qtp = ctx.enter_context(tc.tile_pool(name="qtp", bufs=2))
    vilp = ctx.enter_context(tc.tile_pool(name="vilp", bufs=2))
    expp = ctx.enter_context(tc.tile_pool(name="expp", bufs=6))
    attnp = ctx.enter_context(tc.tile_pool(name="attnp", bufs=2))
    rsp = ctx.enter_context(tc.tile_pool(name="rsp", bufs=2))
    outp = ctx.enter_context(tc.tile_pool(name="outp", bufs=2))
    pmm = ctx.enter_context(tc.tile_pool(name="pmm", bufs=3, space="PSUM"))
    pu = ctx.enter_context(tc.tile_pool(name="pu", bufs=2, space="PSUM"))

    # ---- constants: ONE packed DMA ----
    wpack = const.tile([D, PACKW], BF16, tag="wpack")
    nc.sync.dma_start(wpack[:], P["wpack"][:])
    wqt = [wpack[:, 0:128], wpack[:, 128:256]]
    wkt = [wpack[:, 256:384], wpack[:, 384:512]]
    wot = [wpack[:, 512:640], wpack[:, 640:768]]
    hspread = wpack[:, 768:896]
    wvt_pad = wpack[:, 896:1152]
    c256 = wpack[:, 1152:1408]
    # per-partition bias columns (spread layouts, f32 for tensor_scalar)
    bpack = const.tile([D, 8], F32, tag="bpack")
    nc.sync.dma_start(bpack[:], P["bpack"][:])
    bq_col = [bpack[:, 0:1], bpack[:, 1:2]]
    bk_col = [bpack[:, 2:3], bpack[:, 3:4]]
    bfin_col = bpack[:, 4:5]

    zbias = const.tile([D, 1], F32, tag="zbias")
    nc.vector.memset(zbias[:], 0.0)

    Exp = mybir.ActivationFunctionType.Exp
    ADD = mybir.AluOpType.add

    def load_proj_pieces(s):
        """Generator: emits load + projections for slab s in small pieces."""
        xv = inp.tile([D, N], BF16, tag="xv", name=f"xv{s}")
        nc.sync.dma_start(xv[:], P["xv"][s])
        xq = inp.tile([D, N], BF16, tag="xq", name=f"xq{s}")
        nc.sync.dma_start(xq[:], P["xq"][s])
        xk = inp.tile([D, N], BF16, tag="xk", name=f"xk{s}")
        nc.sync.dma_start(xk[:], P["xk"][s])
        vil = vilp.tile([D, 8 * 256], BF16, tag="vil", name=f"vil{s}")
        qt, kt = [], []
        yield (vil, qt, kt)
        for mc in range(8):
            ps = pmm.tile([D, N], F32, tag="mm", name=f"vp{s}_{mc}")
            nc.tensor.matmul(ps[:, 0:256], xv[:, mc * 128 : (mc + 1) * 128],
                             wvt_pad, start=True, stop=True)
            nc.vector.tensor_add(vil[:, mc * 256 : (mc + 1) * 256],
                                 ps[:, 0:256], c256)
            yield None
        for g in range(2):
            for (wt, bcol, xin, dst, tg) in (
                (wqt[g], bq_col[g], xq, qt, f"q{g}"),
                (wkt[g], bk_col[g], xk, kt, f"k{g}"),
            ):
                ps = pmm.tile([D, N], F32, tag="mm", name=f"pj{s}{tg}")
                for nh in range(2):
                    nc.tensor.matmul(ps[:, nh * 512 : (nh + 1) * 512], wt,
                                     xin[:, nh * 512 : (nh + 1) * 512],
                                     start=True, stop=True)
                t = qtp.tile([D, N], BF16, tag=tg, name=f"t{s}{tg}")
                nc.vector.tensor_scalar(t[:], ps[:], bcol, None, ADD)
                dst.append(t)
                yield None
        while True:
            yield None

    def attention_group(s, g, vil, qt, kt, interleave, fills=()):
        fills = list(fills)
        u = [pu.tile([D, 512], F32, tag="u", name=f"u{s}{g}_{nh}")
             for nh in range(2)]
        pend_pv = None

        def emit_pv(mc, exs, last_exp):
            for nh in range(2):
                for j in range(4):
                    lo = mc * 256 + g * 128 + 32 * j
                    mm = nc.tensor.matmul(
                        u[nh][32 * j : 32 * j + 32, :],
                        vil[:, lo : lo + 32],
                        exs[j][:, nh * 512 : (nh + 1) * 512],
                        start=(mc == 0), stop=(mc == 7),
                        tile_position=(0, 32 * j))
                    add_dep_helper(mm.ins, last_exp.ins,
                                   reason="PV quad grouping")

        for mc in range(8):
            if mc >= 2 and fills:
                fills.pop(0)()
            exs, ex_insts = [], []
            for j in range(4):
                sc = pmm.tile([D, N], F32, tag="mm", name=f"sc{s}{g}{mc}_{j}")
                for nh in range(2):
                    nc.tensor.matmul(
                        sc[:, nh * 512 : (nh + 1) * 512],
                        kt[g][32 * j : 32 * j + 16, mc * 128 : (mc + 1) * 128],
                        qt[g][32 * j : 32 * j + 16, nh * 512 : (nh + 1) * 512],
                        start=True, stop=True, tile_position=(32 * j, 0),
                    )
                ex = expp.tile([D, N], BF16, tag="ex", name=f"ex{s}{g}{mc}_{j}")
                ei = nc.scalar.activation(ex[:], sc[:], Exp, bias=zbias[:, 0:1],
                                          scale=float(SCALE))
                exs.append(ex)
                ex_insts.append(ei)
                # defer the previous mc's PV octet until two QK pairs of
                # this mc are in the PE stream (fills the ACT-lag window)
                if j == 1 and pend_pv is not None:
                    pend_pv()
                    pend_pv = None
            pend_pv = (lambda mc=mc, exs=exs, le=ex_insts[3]:
                       emit_pv(mc, exs, le))
            if interleave is not None:
                next(interleave)
                next(interleave)
        pend_pv()
        return u

    def norm_dve(s, g, u):
        # copy U out of PSUM first so the pool slots free early (DVE only)
        uc = rsp.tile([D, N], F32, tag="uc", name=f"uc{s}{g}")
        nc.vector.tensor_copy(uc[:, 0:512], u[0][:])
        nc.vector.tensor_copy(uc[:, 512:1024], u[1][:])
        rtmp = rsp.tile([D, N], F32, tag="rtmp", name=f"rt{s}{g}")
        nc.vector.reciprocal_approx_fast(rtmp[:], uc[:])
        rrec = rsp.tile([D, N], BF16, tag="rrec", name=f"rr{s}{g}")
        nc.vector.tensor_copy(rrec[:], rtmp[:])
        return uc, rrec

    def norm_pe(s, g, uc, rrec):
        # spread matmul + normalize; emitted where the PE has ready work
        a = attnp.tile([D, N], BF16, tag=f"at{g}", name=f"a{s}{g}")
        rps = pmm.tile([D, N], F32, tag="mm", name=f"rp{s}{g}")
        for nh in range(2):
            nc.tensor.matmul(rps[:, nh * 512 : (nh + 1) * 512], hspread,
                             rrec[:, nh * 512 : (nh + 1) * 512],
                             start=True, stop=True)
        rsb = rsp.tile([D, N], F32, tag="rsb", name=f"rb{s}{g}")
        nc.vector.tensor_copy(rsb[:], rps[:])
        nc.vector.tensor_mul(a[:], uc[:], rsb[:])
        return a

    def final_out(s, at):
        fin = pmm.tile([D, N], F32, tag="mm", name=f"fin{s}")
        for nh in range(2):
            c = fin[:, nh * 512 : (nh + 1) * 512]
            nc.tensor.matmul(c, wot[0], at[0][:, nh * 512 : (nh + 1) * 512],
                             start=True, stop=False)
            nc.tensor.matmul(c, wot[1], at[1][:, nh * 512 : (nh + 1) * 512],
                             start=False, stop=True)
        ot = outp.tile([D, N], F32, tag="ot", name=f"ot{s}")
        nc.vector.tensor_scalar(ot[:], fin[:], bfin_col, None, ADD)
        nc.sync.dma_start(P["out"][s], ot[:])

    pipe = load_proj_pieces(0)
    cur = next(pipe)
    for _ in range(21):
        next(pipe)
    carry = []  # fills deferred into the next slab's attention-A
    for s in range(SLABS):
        vil, qt, kt = cur
        nxt_pipe = load_proj_pieces(s + 1) if s + 1 < SLABS else None
        nxt = next(nxt_pipe) if nxt_pipe else None

        slabfills = {}
        u_a = attention_group(s, 0, vil, qt, kt, None, fills=carry)
        carry = []
        uc_a, rrec_a = norm_dve(s, 0, u_a)
        at_s = []
        fills_b = [lambda s=s, uc=uc_a, rr=rrec_a: at_s.append(norm_pe(s, 0, uc, rr))]
        u_b = attention_group(s, 1, vil, qt, kt, nxt_pipe, fills=fills_b)
        uc_b, rrec_b = norm_dve(s, 1, u_b)

        def mk_tail(s, uc_b, rrec_b, at_s):
            def f1():
                at_s.append(norm_pe(s, 1, uc_b, rrec_b))
            def f2():
                final_out(s, at_s)
            return [f1, f2]
        carry = mk_tail(s, uc_b, rrec_b, at_s)
        cur = nxt
    # last slab's tail has no next attention block to hide in
    for f in carry:
        f()


_CACHE: dict = {}


def _get_nc():
    if "nc" in _CACHE:
        return _CACHE["nc"]
    nc = bacc.Bacc()
    P = {}
    for name, shape in (
        ("xq", (SLABS, D, N)), ("xk", (SLABS, D, N)), ("xv", (SLABS, D, N)),
        ("wpack", (D, PACKW)),
    ):
        P[name] = nc.declare_dram_parameter(name, list(shape), BF16, isOutput=False)
    P["bpack"] = nc.declare_dram_parameter("bpack", [D, 8], F32, isOutput=False)
    P["out"] = nc.declare_dram_parameter("out", [SLABS, D, N], F32, isOutput=True)

    with tile.TileContext(nc) as tc:
        _build_kernel(tc, P)
    nc.finalize()
    _CACHE["nc"] = nc
    return nc


def _spread_w(W, off):
    """(128,128) lhsT for q/k projection: head j of this group at cols 32j."""
    A = np.zeros((D, D), np.float32)
    for j in range(4):
        A[:, 32 * j : 32 * j + 16] = W[off + 16 * j : off + 16 * j + 16, :].T
    return A


def _spread_b(b, off):
    r = np.zeros(D, np.float32)
    for j in range(4):
        r[32 * j : 32 * j + 16] = b[off + 16 * j : off + 16 * j + 16]
    return r


def _host_consts(Wq, bq, Wk, bk, Wv, bv, Wo, bo):
    pack = np.zeros((D, PACKW), np.float32)
    pack[:, 0:128] = _spread_w(Wq, 0)
    pack[:, 128:256] = _spread_w(Wq, 64)
    pack[:, 256:384] = _spread_w(Wk, 0)
    pack[:, 384:512] = _spread_w(Wk, 64)
    wo_a = np.zeros((D, D), np.float32)
    wo_b = np.zeros((D, D), np.float32)
    for j in range(4):
        wo_a[32 * j : 32 * j + 16, :] = Wo[:, 16 * j : 16 * j + 16].T
        wo_b[32 * j : 32 * j + 16, :] = Wo[:, 64 + 16 * j : 64 + 16 * j + 16].T
    pack[:, 512:640] = wo_a
    pack[:, 640:768] = wo_b
    hs = np.zeros((D, D), np.float32)
    for p in range(D):
        hs[32 * (p // 32) + 16, p] = 1.0
    pack[:, 768:896] = hs
    wvt = np.zeros((D, 256), np.float32)
    c256 = np.zeros((D, 256), np.float32)
    for g in range(2):
        for j in range(4):
            h = 4 * g + j
            base = g * 128 + 32 * j
            wvt[:, base : base + 16] = Wv[16 * h : 16 * h + 16, :].T
            c256[:, base + 16 : base + 32] = 1.0
    pack[:, 896:1152] = wvt
    pack[:, 1152:1408] = c256
    bp = np.zeros((D, 8), np.float32)
    bp[:, 0] = _spread_b(bq, 0)
    bp[:, 1] = _spread_b(bq, 64)
    bp[:, 2] = _spread_b(bk, 0)
    bp[:, 3] = _spread_b(bk, 64)
    bp[:, 4] = (Wo @ bv + bo).astype(np.float32)
    import ml_dtypes
    return {"wpack": pack.astype(ml_dtypes.bfloat16), "bpack": bp}


def kernel(**inputs) -> np.ndarray:
    q = np.asarray(inputs["query"], np.float32)
    k = np.asarray(inputs["key"], np.float32)
    v = np.asarray(inputs["value"], np.float32)
    consts = _host_consts(
        *(np.asarray(inputs[n], np.float32)
          for n in ("Wq", "bq", "Wk", "bk", "Wv", "bv", "Wo", "bo"))
    )
    # slabs in (D, N) layout, bf16 for full-rate PE streams
    import ml_dtypes
    bf = ml_dtypes.bfloat16
    qT = np.ascontiguousarray(q.reshape(B * T, N, D).transpose(0, 2, 1)).astype(bf)
    kT = np.ascontiguousarray(k.reshape(B * T, N, D).transpose(0, 2, 1)).astype(bf)
    vT = np.ascontiguousarray(v.reshape(B * T, N, D).transpose(0, 2, 1)).astype(bf)

    nc = _get_nc()
    in_maps = []
    for c in range(NCORES):
        sl = slice(SLABS * c, SLABS * (c + 1))
        m = {"xq": qT[sl], "xk": kT[sl], "xv": vT[sl]}
        m.update(consts)
        in_maps.append(m)

    res = run_bass_kernel_spmd(nc, in_maps, core_ids=list(range(NCORES)),
                               trace=bool(int(os.environ.get("KERNEL_TRACE", "0"))))
    _CACHE["last_result"] = res
    out = np.concatenate([res.results[c]["out"] for c in range(NCORES)], axis=0)
    return np.ascontiguousarray(
        out.transpose(0, 2, 1).reshape(B, T, N, D)).astype(np.float32)
